# revision 12
# baseline (speedup 1.0000x reference)
"""MoE top-k routing + grouped down-proj GEMM + reduce-scatter for trn2 (8 cores).

Problem: intermediate_states [4, 2048, 1024] f16 (rank-sharded expanded-token
activations), w [4, 8, 1024, 2048] f16 (rank-sharded per-expert down-proj),
router_logits [1024, 8] f32, topk=2.  Output [4, 256, 2048] f16.

Strategy: per expanded token tk routed to expert e(tk):
y_part[tk] = gate(tk) * (x_full[tk] @ w_full[e(tk)]) with x_full [TK, 4096]
(rank dim folded into the contraction) and w_full[e] [4096, 2048].

Work is decomposed into (expert, K-half) groups; a group's tokens are split
into 32-token chunks (sum over groups = ~136 chunks for balanced routing vs
160 128-token-padded quarters in a 5-job layout).  Each core holds two W
slices (A, B: one (expert, khalf) [2048, 2048] f16 block each) and runs
4 full tiles + 1 half tile of PE work using 4x column tiling
(tile_size=(128, 32)): each tile issues 4 concurrent matmuls per (ks, nf)
group, one per 32-token column chunk, each streaming its own W slice.
Column capacity per core: 10 chunk-slots on slice A, 8 on slice B
(the half tile carries 2 B-chunks split into (ks 0-7)/(ks 8-15) halves).
The host pairs the 16 (e, kh) groups onto cores (largest with smallest),
which fits whenever the largest group is <= 10 chunks and the 9th largest
is <= 8.  PE work per core: 4.5 tile-equivalents (= 288 4-way column
groups + half-tile) ~= 62-66 us vs 69 us for the 5-job layout.

Each chunk accumulates fp32 in its own PSUM quarter region over its k-run
and gets its fp32 routing gate applied as a per-partition scale at PSUM
eviction (scalar engine for nf 0-1, vector for nf 2-3).  The final (half)
tile runs nf-outer so its quarters evict pipelined, shrinking the kernel
tail.  No collective: the host sums each token's partial rows.

Fallback: pathological routing (largest group > 10 chunks etc.) uses an
expert-per-core kernel (full K=4096, capacity padded to 128).
"""

import numpy as np

R, T_TOK, TOPK, E = 4, 1024, 2, 8
I_PR, H = 1024, 2048
K = R * I_PR            # 4096 contraction
P = 128
NF = 512                # matmul free-dim (one PSUM bank of fp32)
NH = H // NF            # 4
N_CORES = 8

KH = K // 2             # 2048 per K-half
KS2 = KH // P           # 16 k-subtiles per K-half
CH = 32                 # token chunk granularity (column-tile width)
NFULL = 4               # full tiles per core (+1 half tile)
NTILE = NFULL + 1
CAP_A, CAP_B = 10, 8    # chunk-slot capacity per W slice
# compiled slot -> W slice map for the 4 full tiles (0=A, 1=B); the half
# tile is all slice A and runs between phase 1 (t0/t1) and phase 2 (t2/t3),
# which delays the first wB consumption by the half tile's span.
SLICE_OF = ((0, 0, 0, 0), (0, 0, 0, 0), (1, 1, 1, 1), (1, 1, 1, 1))
# half tile: 4 column slots = 2 A-chunks split into k-halves
HALF_KOFF = (0, 8, 0, 8)
NWARM = 24

# fallback (expert-per-core) mode
KSUB = K // P           # 32
CAP_FB = 384            # token capacity per launch in fallback mode

_prog_cache: dict[str, object] = {}


def _new_bacc():
    from concourse import bacc

    return bacc.Bacc(
        "TRN2",
        target_bir_lowering=False,
        debug=False,
        num_devices=N_CORES,
    )


def _build_program_tiles():
    import concourse.mybir as mybir
    import concourse.tile as tile

    f16 = mybir.dt.float16
    f32 = mybir.dt.float32

    nc = _new_bacc()
    # xj[t, p, ks*P + c*CH + m] = x value of tile-t column-chunk c token m at
    # K-row ks*P + p of the chunk's K-half: the SBUF stationary layout.
    xj = nc.declare_dram_parameter("xj", [NFULL, P, KS2 * P], f16, isOutput=False)
    xh = nc.declare_dram_parameter("xh", [P, 8 * P], f16, isOutput=False)
    wh = nc.declare_dram_parameter("wh", [2, KS2, P, H], f16, isOutput=False)
    gs = nc.declare_dram_parameter("gs", [P, NTILE], f32, isOutput=False)
    ho = nc.declare_dram_parameter("ho", [NTILE, P, H], f16, isOutput=True)

    with tile.TileContext(nc) as tc:
        with tc.tile_pool(name="sb", bufs=1) as sb, \
             tc.tile_pool(name="ps", bufs=2, space="PSUM") as psp:
            xt = [sb.tile([P, KS2 * P], f16, name=f"x{t}", tag=f"x{t}", bufs=1)
                  for t in range(NFULL)]
            xh_t = sb.tile([P, 8 * P], f16, name="xh", tag="xh", bufs=1)
            wt = [[sb.tile([P, H], f16, name=f"w{s}_{ks}", tag=f"w{s}_{ks}",
                           bufs=1) for ks in range(KS2)] for s in range(2)]
            g_raw = sb.tile([P, NTILE], f32, name="g_raw", tag="g_raw", bufs=1)

            HXB = KS2 * P // 2  # half of a full x tile's free dim

            def dma_x(t, half):
                sl = slice(half * HXB, (half + 1) * HXB)
                nc.sync.dma_start(xt[t][:, sl], xj[t, :, sl])

            def dma_w(s, ks, eng=None):
                (eng or nc.sync).dma_start(wt[s][ks][:], wh[s, ks, :, :])

            # Input DMAs ride two HW queues (sync + scalar): aggregate
            # delivery is ~1.3x a single queue's.  Each ring drains in
            # order, so both lists are laid out so per-queue cumulative
            # arrival (at ~half aggregate bandwidth) stays ahead of the
            # interleaved consumption order; wA0 is split across both
            # queues so the first matmul group starts earliest.
            def dma_xsc(t, half):
                sl = slice(half * HXB, (half + 1) * HXB)
                nc.scalar.dma_start(xt[t][:, sl], xj[t, :, sl])

            HWB = H // 2

            # scalar queue: wA0 second half + odd W chunks + xh + late x.
            nc.scalar.dma_start(wt[0][0][:, HWB:], wh[0, 0, :, HWB:])
            dma_xsc(1, 0)
            nc.scalar.dma_start(g_raw[:], gs[:, :])
            dma_w(0, 1, nc.scalar)
            dma_w(0, 3, nc.scalar)
            dma_xsc(0, 1)
            dma_w(0, 5, nc.scalar)
            dma_xsc(1, 1)
            for ks in range(7, KS2, 2):
                dma_w(0, ks, nc.scalar)
            nc.scalar.dma_start(xh_t[:], xh[:, :])
            dma_w(1, 1, nc.scalar)
            dma_w(1, 3, nc.scalar)
            dma_w(1, 5, nc.scalar)
            dma_xsc(3, 1)
            for ks in range(7, KS2, 2):
                dma_w(1, ks, nc.scalar)
            # sync queue: wA0 first half + even W chunks + remaining x.
            nc.sync.dma_start(wt[0][0][:, :HWB], wh[0, 0, :, :HWB])
            dma_x(0, 0)
            for ks in range(2, KS2, 2):
                dma_w(0, ks)
            dma_x(2, 0)
            dma_x(3, 0)
            dma_w(1, 0)
            dma_w(1, 2)
            dma_w(1, 4)
            dma_w(1, 6)
            dma_x(2, 1)
            for ks in range(8, KS2, 2):
                dma_w(1, ks)

            # gates: a scalar-engine copy of g_raw; evictions read the copy
            # so their gate dependency is ACT-engine-local.
            g2 = sb.tile([P, NTILE], f32, name="g2", tag="g2", bufs=1)
            nc.scalar.copy(g2[:], g_raw[:])

            psq = {}

            def open_tile(t):
                psq[t] = [psp.tile([P, NF], f32, name=f"ps{t}_{nf}", tag="ps",
                                   bufs=8) for nf in range(NH)]

            open_tile(0)
            open_tile(1)

            # HAM warmup in the same (128, 32) tile mode as the real matmuls:
            # keeps the PE busy while the first DMAs stream in; garbage goes
            # to tile 0's first PSUM quarter, cleared by the first real
            # start=True matmul.
            warm_in = sb.tile([P, P], f16, name="warm_in", tag="warm", bufs=1)
            nc.vector.memset(warm_in[:], 0.0)
            for i in range(NWARM):
                nc.tensor.matmul(
                    psq[0][0][0:CH, 0:P],
                    lhsT=warm_in[:, 0:CH],
                    rhs=warm_in[:],
                    start=(i == 0),
                    stop=(i == NWARM - 1),
                    tile_position=(0, 0),
                )

            def mm_group(t, ks):
                for nf in range(NH):
                    for c in range(4):
                        s = SLICE_OF[t][c]
                        nc.tensor.matmul(
                            psq[t][nf][c * CH:(c + 1) * CH, :],
                            lhsT=xt[t][:, ks * P + c * CH:ks * P + (c + 1) * CH],
                            rhs=wt[s][ks][:, nf * NF:(nf + 1) * NF],
                            start=(ks == 0),
                            stop=(ks == KS2 - 1),
                            tile_position=(0, c * CH),
                        )

            def evict_quarter(t, nf, o_t):
                dst = o_t[:, nf * NF:(nf + 1) * NF]
                src = psq[t][nf][:]
                if nf >= 2:
                    nc.vector.tensor_scalar_mul(dst, src, g2[:, t:t + 1])
                else:
                    nc.scalar.activation(
                        dst, src,
                        mybir.ActivationFunctionType.Copy,
                        scale=g2[:, t:t + 1],
                    )

            def dma_out(t, o_t, half):
                nc.sync.dma_start(
                    ho[t, :, half * (H // 2):(half + 1) * (H // 2)],
                    o_t[:, half * (H // 2):(half + 1) * (H // 2)])

            def evict_pair(ta, tb):
                o_a = sb.tile([P, H], f16, name=f"o{ta}", tag="o", bufs=NTILE)
                o_b = sb.tile([P, H], f16, name=f"o{tb}", tag="o", bufs=NTILE)
                # ta stopped first (de-interleaved phase tail); pipeline both
                # engines and the output DMAs per half.
                evict_quarter(ta, 0, o_a)
                evict_quarter(ta, 2, o_a)
                evict_quarter(ta, 1, o_a)
                evict_quarter(ta, 3, o_a)
                evict_quarter(tb, 0, o_b)
                evict_quarter(tb, 2, o_b)
                dma_out(ta, o_a, 0)
                dma_out(ta, o_a, 1)
                evict_quarter(tb, 1, o_b)
                evict_quarter(tb, 3, o_b)
                dma_out(tb, o_b, 0)
                dma_out(tb, o_b, 1)

            def phase(ta, tb):
                # interleaved per k-subtile (the pair consumes each arriving
                # W chunk over ~1.8us, matching the DMA stream rate); the
                # last two k-subtiles de-interleave so ta's eviction overlaps
                # tb's final matmuls.
                for ks in range(KS2 - 2):
                    for t in (ta, tb):
                        mm_group(t, ks)
                for t in (ta, tb):
                    for ks in (KS2 - 2, KS2 - 1):
                        mm_group(t, ks)
                evict_pair(ta, tb)

            # phase 1: tiles 0+1 on slice A.
            phase(0, 1)

            # phase 1.5: half tile on resident slice A (its span defers the
            # first wB consumption, buying the B stream ~7us of headroom).
            # nf-outer so its quarters evict pipelined.
            o_h = sb.tile([P, H], f16, name="o4", tag="o", bufs=NTILE)
            for nf in range(NH):
                q = psp.tile([P, NF], f32, name=f"ps4_{nf}", tag="ps", bufs=8)
                for ks in range(8):
                    for c in range(4):
                        nc.tensor.matmul(
                            q[c * CH:(c + 1) * CH, :],
                            lhsT=xh_t[:, ks * P + c * CH:ks * P + (c + 1) * CH],
                            rhs=wt[0][HALF_KOFF[c] + ks][:, nf * NF:(nf + 1) * NF],
                            start=(ks == 0),
                            stop=(ks == 7),
                            tile_position=(0, c * CH),
                        )
                dst = o_h[:, nf * NF:(nf + 1) * NF]
                if nf % 2:
                    nc.vector.tensor_scalar_mul(dst, q[:], g2[:, 4:5])
                else:
                    nc.scalar.activation(
                        dst, q[:],
                        mybir.ActivationFunctionType.Copy,
                        scale=g2[:, 4:5],
                    )
                if nf == 1:
                    dma_out(NFULL, o_h, 0)
                elif nf == 3:
                    dma_out(NFULL, o_h, 1)

            # phase 2: tiles 2+3 on slice B.
            open_tile(2)
            open_tile(3)
            phase(2, 3)
    nc.finalize()
    return nc


def _build_program_fallback(cap: int):
    import concourse.mybir as mybir
    import concourse.tile as tile

    f16 = mybir.dt.float16
    f32 = mybir.dt.float32
    ntok = cap // P

    nc = _new_bacc()
    xT = nc.declare_dram_parameter("xT", [KSUB, P, cap], f16, isOutput=False)
    wk = nc.declare_dram_parameter("wk", [KSUB, P, H], f16, isOutput=False)
    gs = nc.declare_dram_parameter("gs", [P, ntok], f32, isOutput=False)
    ho = nc.declare_dram_parameter("ho", [ntok, P, H], f16, isOutput=True)

    with tile.TileContext(nc) as tc:
        with tc.tile_pool(name="sb", bufs=1) as sb, \
             tc.tile_pool(name="ps", bufs=2, space="PSUM") as psp:
            xt, wt = [], []
            for k in range(KSUB):
                x_t = sb.tile([P, cap], f16, name=f"x{k}", tag=f"x{k}", bufs=1)
                nc.sync.dma_start(x_t[:], xT[k, :, :])
                w_t = sb.tile([P, H], f16, name=f"w{k}", tag=f"w{k}", bufs=1)
                nc.sync.dma_start(w_t[:], wk[k, :, :])
                xt.append(x_t)
                wt.append(w_t)
            g_raw = sb.tile([P, ntok], f32, name="g_raw", tag="g_raw", bufs=1)
            nc.sync.dma_start(g_raw[:], gs[:, :])
            g2 = sb.tile([P, ntok], f32, name="g2", tag="g2", bufs=1)
            nc.scalar.copy(g2[:], g_raw[:])

            for t in range(ntok):
                ps = psp.tile([P, H], f32, name=f"ps{t}", tag="ps", bufs=2)
                for k in range(KSUB):
                    lhs = xt[k][:, t * P:(t + 1) * P]
                    for h in range(NH):
                        nc.tensor.matmul(
                            ps[:, h * NF:(h + 1) * NF],
                            lhsT=lhs,
                            rhs=wt[k][:, h * NF:(h + 1) * NF],
                            start=(k == 0),
                            stop=(k == KSUB - 1),
                        )
                o_t = sb.tile([P, H], f16, name=f"o{t}", tag="o", bufs=ntok)
                nc.scalar.activation(
                    o_t[:],
                    ps[:],
                    mybir.ActivationFunctionType.Copy,
                    scale=g2[:, t:t + 1],
                )
                nc.sync.dma_start(ho[t, :, :], o_t[:])
    nc.finalize()
    return nc


def _get_program(key):
    if key not in _prog_cache:
        if key == "tiles":
            _prog_cache[key] = _build_program_tiles()
        else:
            _prog_cache[key] = _build_program_fallback(int(key.split(":")[1]))
    return _prog_cache[key]


def _route(logits, topk):
    """numpy replica of jax.lax.top_k + softmax over selected logits."""
    idx = np.argsort(-logits, axis=-1, kind="stable")[:, :topk]      # [T, topk]
    vals = np.take_along_axis(logits, idx, axis=-1)
    mx = vals.max(-1, keepdims=True)
    gate = np.exp(vals - mx)
    gate = gate / gate.sum(-1, keepdims=True)                        # f32
    return idx, gate


def _pair_groups(chunk_counts):
    """Pair the 16 (e, kh) groups onto 8 cores: i-th largest with i-th
    smallest.  Returns [(groupA, groupB)] or None if some pair exceeds the
    compiled (CAP_A, CAP_B) chunk-slot capacity."""
    groups = []
    for e, n in enumerate(chunk_counts):
        for kh in range(2):
            groups.append((int(n), e, kh))
    groups.sort(reverse=True)
    pairs = []
    for i in range(N_CORES):
        ga, gb = groups[i], groups[15 - i]
        if ga[0] > CAP_A or gb[0] > CAP_B:
            return None
        pairs.append((ga, gb))
    return pairs


def prepare(inputs):
    """Host routing + per-core input construction.

    Returns (nc, launches, combine): launches is a list of per-launch in_maps
    (one dict per core); combine(list_of_per_launch_results) -> final output.
    """
    x = np.asarray(inputs["intermediate_states"])          # [R, TK, I_PR] f16
    w = np.asarray(inputs["w"])                            # [R, E, I_PR, H] f16
    logits = np.asarray(inputs["router_logits"]).astype(np.float32)  # [T, E]
    topk = int(np.asarray(inputs["topk"]))

    T, E_ = logits.shape
    TK = T * topk
    assert x.shape == (R, TK, I_PR) and w.shape == (R, E_, I_PR, H) and E_ == E

    idx, gate = _route(logits, topk)
    flat_e = idx.reshape(-1)                               # expert of tk
    counts = np.bincount(flat_e, minlength=E)
    starts = np.zeros(E + 1, np.int64)
    starts[1:] = np.cumsum(counts)
    order = np.argsort(flat_e, kind="stable")              # tks sorted by expert
    g_flat = gate.reshape(TK)
    xf = np.ascontiguousarray(x.transpose(1, 0, 2)).reshape(TK, K)  # [TK, 4096]

    chunk_counts = [-(-int(c) // CH) for c in counts]
    pairs = _pair_groups(chunk_counts)
    if pairs is not None:
        return _prepare_tiles(w, xf, g_flat, order, starts, pairs, topk, T)
    return _prepare_fallback(w, xf, g_flat, order, starts, counts, topk, T)


# chunk-slot order per W slice: (tile, col) positions; A overflows into the
# half tile (2 chunks, k-split across column pairs)
A_SLOTS = [(0, 0), (0, 1), (0, 2), (0, 3), (1, 0), (1, 1), (1, 2), (1, 3)]
B_SLOTS = [(2, 0), (2, 1), (2, 2), (2, 3), (3, 0), (3, 1), (3, 2), (3, 3)]


def _prepare_tiles(w, xf, g_flat, order, starts, pairs, topk, T):
    TK = T * topk
    nc = _get_program("tiles")

    xjs = np.zeros((N_CORES, NFULL, P, KS2, P), np.float16)
    xhs = np.zeros((N_CORES, P, 8, P), np.float16)
    whs = np.zeros((N_CORES, 2, KS2, P, H), np.float16)
    gss = np.zeros((N_CORES, P, NTILE), np.float32)
    # pos[plane, tk] = row index of tk's partial in the assembled h rows;
    # planes 0/1 = kh 0/1 main partial, 2/3 = kh 0/1 half-tile second half.
    ZROW = N_CORES * NTILE * P
    pos = np.full((4, TK), ZROW, np.int64)

    for core, (ga, gb) in enumerate(pairs):
        for s, (nch, e, kh) in enumerate((ga, gb)):
            if nch == 0:
                continue
            toks_e = order[starts[e]:starts[e + 1]]
            whs[core, s] = np.ascontiguousarray(
                w[2 * kh:2 * kh + 2, e].reshape(KH, H)).reshape(KS2, P, H)
            slots = A_SLOTS if s == 0 else B_SLOTS
            for j in range(nch):
                toks = toks_e[j * CH:(j + 1) * CH]
                n = len(toks)
                xs = xf[toks, kh * KH:(kh + 1) * KH]       # [n, 2048] f16
                blk = xs.reshape(n, KS2, P).transpose(2, 1, 0)  # [P, ks, n]
                if j < len(slots):
                    t, c = slots[j]
                    xjs[core, t, :, :, c * CH:c * CH + n] = blk
                    gss[core, c * CH:c * CH + n, t] = g_flat[toks]
                    pos[kh, toks] = (core * NTILE + t) * P + c * CH + np.arange(n)
                else:
                    # half tile: chunk split into (ks 0-7, ks 8-15) columns
                    jj = j - len(slots)
                    assert s == 0 and jj < 2
                    for h in range(2):
                        c = 2 * jj + h
                        xhs[core, :, :, c * CH:c * CH + n] = blk[:, 8 * h:8 * (h + 1), :]
                        gss[core, c * CH:c * CH + n, NFULL] = g_flat[toks]
                        pos[2 * h + kh, toks] = \
                            (core * NTILE + NFULL) * P + c * CH + np.arange(n)

    launches = [[{"xj": xjs[c].reshape(NFULL, P, KS2 * P),
                  "xh": xhs[c].reshape(P, 8 * P),
                  "wh": whs[c], "gs": gss[c]} for c in range(N_CORES)]]

    def combine(all_results):
        res = all_results[0]
        h_all = np.concatenate(
            [res[c]["ho"].reshape(NTILE * P, H) for c in range(N_CORES)]
            + [np.zeros((1, H), np.float16)], axis=0)
        y = np.zeros((T, H), np.float32)
        for plane in range(4):
            for kk in range(topk):
                y += h_all[pos[plane, kk::topk]].astype(np.float32)
        return y.astype(np.float16).reshape(R, T // R, H)

    return nc, launches, combine


def _prepare_fallback(w, xf, g_flat, order, starts, counts, topk, T):
    TK = T * topk
    cap_needed = -(-max(int(counts.max()), 1) // P) * P
    cap_launch = min(cap_needed, CAP_FB)
    n_launch = -(-cap_needed // cap_launch)
    cap_total = n_launch * cap_launch
    ntok_l = cap_launch // P

    nc = _get_program(f"fb:{cap_launch}")

    pos = np.empty(TK, np.int64)
    for e in range(E):
        toks = order[starts[e]:starts[e + 1]]
        pos[toks] = e * cap_total + np.arange(len(toks))

    launches = []
    for j in range(n_launch):
        in_maps = []
        for e in range(E):
            toks = order[starts[e]:starts[e + 1]][j * cap_launch:(j + 1) * cap_launch]
            c = len(toks)
            xTe = np.zeros((K, cap_launch), np.float16)
            gse = np.zeros((cap_launch,), np.float32)
            if c:
                xTe[:, :c] = xf[toks].T
                gse[:c] = g_flat[toks]
            in_maps.append({
                "xT": np.ascontiguousarray(xTe.reshape(KSUB, P, cap_launch)),
                "wk": np.ascontiguousarray(w[:, e].reshape(K, H)).reshape(KSUB, P, H),
                "gs": np.ascontiguousarray(gse.reshape(ntok_l, P).T),
            })
        launches.append(in_maps)

    def combine(all_results):
        h_all = np.empty((E * cap_total, H), np.float16)
        for j, res in enumerate(all_results):
            for e in range(E):
                h_all[e * cap_total + j * cap_launch:
                      e * cap_total + (j + 1) * cap_launch] = \
                    res[e]["ho"].reshape(cap_launch, H)
        y = h_all[pos[0::topk]].astype(np.float32)
        for kk in range(1, topk):
            y += h_all[pos[kk::topk]].astype(np.float32)
        return y.astype(np.float16).reshape(R, T // R, H)

    return nc, launches, combine


def kernel(**inputs) -> np.ndarray:
    nc, launches, combine = prepare(inputs)
    from concourse.bass_utils import run_bass_kernel_spmd

    all_results = []
    for in_maps in launches:
        res = run_bass_kernel_spmd(nc, in_maps, core_ids=list(range(N_CORES)))
        all_results.append(res.results)
    return combine(all_results)


# revision 21
# speedup vs baseline: 1.2758x; 1.2758x over previous
"""MoE top-k routing + grouped down-proj GEMM + reduce-scatter for trn2 (8 cores).

Problem: intermediate_states [4, 2048, 1024] f16 (rank-sharded expanded-token
activations), w [4, 8, 1024, 2048] f16 (rank-sharded per-expert down-proj),
router_logits [1024, 8] f32, topk=2.  Output [4, 256, 2048] f16.

Strategy: per expanded token tk routed to expert e(tk):
y_part[tk] = gate(tk) * (x_full[tk] @ w_full[e(tk)]) with x_full [TK, 4096]
(rank dim folded into the contraction) and w_full[e] [4096, 2048].

Work is decomposed into (expert, K-half) groups; a group's tokens are split
into 32-token chunks (sum over groups = ~136 chunks for balanced routing vs
160 128-token-padded quarters in a 5-job layout).  Each core holds two W
slices (A, B: one (expert, khalf) [2048, 2048] f16 block each) and runs
4 full tiles + 1 half tile of PE work using 4x column tiling
(tile_size=(128, 32)): each tile issues 4 concurrent matmuls per (ks, nf)
group, one per 32-token column chunk, each streaming its own W slice.
Column capacity per core: 10 chunk-slots on slice A, 8 on slice B
(the half tile carries 2 B-chunks split into (ks 0-7)/(ks 8-15) halves).
The host pairs the 16 (e, kh) groups onto cores (largest with smallest),
which fits whenever the largest group is <= 10 chunks and the 9th largest
is <= 8.  PE work per core: 4.5 tile-equivalents (= 288 4-way column
groups + half-tile) ~= 62-66 us vs 69 us for the 5-job layout.

Each chunk accumulates fp32 in its own PSUM quarter region over its k-run
and gets its fp32 routing gate applied as a per-partition scale at PSUM
eviction (scalar engine for nf 0-1, vector for nf 2-3).  The final (half)
tile runs nf-outer so its quarters evict pipelined, shrinking the kernel
tail.  No collective: the host sums each token's partial rows.

Fallback: pathological routing (largest group > 10 chunks etc.) uses an
expert-per-core kernel (full K=4096, capacity padded to 128).
"""

import numpy as np

R, T_TOK, TOPK, E = 4, 1024, 2, 8
I_PR, H = 1024, 2048
K = R * I_PR            # 4096 contraction
P = 128
NF = 512                # matmul free-dim (one PSUM bank of fp32)
NH = H // NF            # 4
N_CORES = 8

KH = K // 2             # 2048 per K-half
KS2 = KH // P           # 16 k-subtiles per K-half
CH = 32                 # token chunk granularity (column-tile width)
NFULL = 4               # full tiles per core (+1 half tile)
NTILE = NFULL + 1
CAP_A, CAP_B = 10, 8    # chunk-slot capacity per W slice
# compiled tile -> W slice map for the 4 full tiles (0=A, 1=B); the half
# tile is all slice A and runs between phase 1 (t0/t1) and phase 2 (t2/t3),
# which delays the first wB consumption by the half tile's span.
SLICE_OF = (0, 0, 1, 1)
# half tile: 4 column slots = 2 A-chunks split into k-halves
HALF_KOFF = (0, 8, 0, 8)
NWARM = 24

# fallback (expert-per-core) mode
KSUB = K // P           # 32
CAP_FB = 384            # token capacity per launch in fallback mode

_prog_cache: dict[str, object] = {}


def _new_bacc():
    from concourse import bacc

    return bacc.Bacc(
        "TRN2",
        target_bir_lowering=False,
        debug=False,
        num_devices=N_CORES,
    )


def _build_program_tiles():
    import concourse.mybir as mybir
    import concourse.tile as tile

    f16 = mybir.dt.float16
    f32 = mybir.dt.float32

    nc = _new_bacc()
    # xj[t, p, ks*P + c*CH + m] = x value of tile-t column-chunk c token m at
    # K-row ks*P + p of the chunk's K-half: the SBUF stationary layout.
    xj = nc.declare_dram_parameter("xj", [NFULL, P, KS2 * P], f16, isOutput=False)
    xh = nc.declare_dram_parameter("xh", [P, 8 * P], f16, isOutput=False)
    wh = nc.declare_dram_parameter("wh", [2, KS2, P, H], f16, isOutput=False)
    gs = nc.declare_dram_parameter("gs", [P, NTILE], f32, isOutput=False)
    ho = nc.declare_dram_parameter("ho", [NTILE, P, H], f16, isOutput=True)

    with tile.TileContext(nc) as tc:
        with tc.tile_pool(name="sb", bufs=1) as sb, \
             tc.tile_pool(name="ps", bufs=2, space="PSUM") as psp:
            xt = [sb.tile([P, KS2 * P], f16, name=f"x{t}", tag=f"x{t}", bufs=1)
                  for t in range(NFULL)]
            xh_t = sb.tile([P, 8 * P], f16, name="xh", tag="xh", bufs=1)
            wt = [[sb.tile([P, H], f16, name=f"w{s}_{ks}", tag=f"w{s}_{ks}",
                           bufs=1) for ks in range(KS2)] for s in range(2)]
            g_raw = sb.tile([P, NTILE], f32, name="g_raw", tag="g_raw", bufs=1)

            HXB = KS2 * P // 2  # half of a full x tile's free dim

            def dma_x(t, half):
                sl = slice(half * HXB, (half + 1) * HXB)
                nc.sync.dma_start(xt[t][:, sl], xj[t, :, sl])

            def dma_w(s, ks, eng=None):
                (eng or nc.sync).dma_start(wt[s][ks][:], wh[s, ks, :, :])

            # All bulk input DMAs ride the sync HW queue in consumption
            # order (a second busy queue lifts aggregate DMA to ~360 GB/s
            # but power-throttles the PE clock to ~2.0 GHz — a net loss).
            # Only three small late-needed x transfers go to the scalar
            # queue to keep the sync ring lean.  wA0 is split in halves so
            # the first matmul group starts as early as possible.
            HWB = H // 2

            nc.sync.dma_start(wt[0][0][:, :HWB], wh[0, 0, :, :HWB])
            dma_x(0, 0)
            nc.sync.dma_start(wt[0][0][:, HWB:], wh[0, 0, :, HWB:])
            dma_x(1, 0)
            nc.sync.dma_start(g_raw[:], gs[:, :])
            dma_w(0, 1)
            dma_w(0, 2)
            dma_x(0, 1)
            dma_w(0, 3)
            dma_x(1, 1)
            for ks in range(4, KS2):
                dma_w(0, ks)
            dma_x(2, 0)
            dma_x(3, 0)
            for ks in range(KS2):
                dma_w(1, ks)

            # gates: a scalar-engine copy of g_raw; evictions read the copy
            # so their gate dependency is ACT-engine-local.
            g2 = sb.tile([P, NTILE], f32, name="g2", tag="g2", bufs=1)
            nc.scalar.copy(g2[:], g_raw[:])
            # three small late-needed x transfers ride the (otherwise idle)
            # scalar HW queue; issued after the g2 copy so they don't steal
            # DMA bandwidth from the critical first W chunks.
            # tiny read of a mid-stream W chunk pins these issues until the
            # A stream's critical window has drained
            scr = sb.tile([1, 1], f16, name="scr", tag="scr", bufs=1)
            nc.scalar.copy(scr[:], wt[0][8][:1, :1])
            nc.scalar.dma_start(xh_t[:], xh[:, :])
            sl31 = slice(HXB, 2 * HXB)
            nc.scalar.dma_start(xt[2][:, sl31], xj[2, :, sl31])
            nc.scalar.dma_start(xt[3][:, sl31], xj[3, :, sl31])

            psq = {}

            def open_tile(t):
                psq[t] = [psp.tile([P, NF], f32, name=f"ps{t}_{nf}", tag="ps",
                                   bufs=8) for nf in range(NH)]

            open_tile(0)
            open_tile(1)

            # HAM warmup in the same (128, 32) tile mode as the real matmuls:
            # keeps the PE busy while the first DMAs stream in; garbage goes
            # to tile 0's first PSUM quarter, cleared by the first real
            # start=True matmul.
            warm_in = sb.tile([P, P], f16, name="warm_in", tag="warm", bufs=1)
            nc.vector.memset(warm_in[:], 0.0)
            for i in range(NWARM):
                nc.tensor.matmul(
                    psq[0][0][:, 0:P],
                    lhsT=warm_in[:],
                    rhs=warm_in[:],
                    start=(i == 0),
                    stop=(i == NWARM - 1),
                )

            # full tiles: all 4 column chunks share one W slice, so each
            # (ks, nf) step is a single M=128 matmul (plain 128x128 mode).
            def mm_group(t, ks):
                s = SLICE_OF[t]
                for nf in range(NH):
                    nc.tensor.matmul(
                        psq[t][nf][:, :],
                        lhsT=xt[t][:, ks * P:(ks + 1) * P],
                        rhs=wt[s][ks][:, nf * NF:(nf + 1) * NF],
                        start=(ks == 0),
                        stop=(ks == KS2 - 1),
                    )

            def evict_quarter(t, nf, o_t):
                dst = o_t[:, nf * NF:(nf + 1) * NF]
                src = psq[t][nf][:]
                if nf >= 2:
                    nc.vector.tensor_scalar_mul(dst, src, g2[:, t:t + 1])
                else:
                    nc.scalar.activation(
                        dst, src,
                        mybir.ActivationFunctionType.Copy,
                        scale=g2[:, t:t + 1],
                    )

            def dma_out(t, o_t, half):
                nc.sync.dma_start(
                    ho[t, :, half * (H // 2):(half + 1) * (H // 2)],
                    o_t[:, half * (H // 2):(half + 1) * (H // 2)])

            def evict_pair(ta, tb):
                o_a = sb.tile([P, H], f16, name=f"o{ta}", tag="o", bufs=NTILE)
                o_b = sb.tile([P, H], f16, name=f"o{tb}", tag="o", bufs=NTILE)
                # ta stopped first (de-interleaved phase tail); pipeline both
                # engines and the output DMAs per half.
                evict_quarter(ta, 0, o_a)
                evict_quarter(ta, 2, o_a)
                evict_quarter(ta, 1, o_a)
                evict_quarter(ta, 3, o_a)
                evict_quarter(tb, 0, o_b)
                evict_quarter(tb, 2, o_b)
                dma_out(ta, o_a, 0)
                dma_out(ta, o_a, 1)
                evict_quarter(tb, 1, o_b)
                evict_quarter(tb, 3, o_b)
                dma_out(tb, o_b, 0)
                dma_out(tb, o_b, 1)

            def phase(ta, tb):
                # interleaved per k-subtile (the pair consumes each arriving
                # W chunk over ~1.8us, matching the DMA stream rate); the
                # last two k-subtiles de-interleave so ta's eviction overlaps
                # tb's final matmuls.
                for ks in range(KS2 - 2):
                    for t in (ta, tb):
                        mm_group(t, ks)
                for t in (ta, tb):
                    for ks in (KS2 - 2, KS2 - 1):
                        mm_group(t, ks)
                evict_pair(ta, tb)

            # phase 1: tiles 0+1 on slice A.
            phase(0, 1)

            # phase 1.5: half tile on resident slice A (its span defers the
            # first wB consumption, buying the B stream ~7us of headroom).
            # nf-outer so its quarters evict pipelined.
            o_h = sb.tile([P, H], f16, name="o4", tag="o", bufs=NTILE)
            for nf in range(NH):
                q = psp.tile([P, NF], f32, name=f"ps4_{nf}", tag="ps", bufs=8)
                for ks in range(8):
                    for c in range(4):
                        nc.tensor.matmul(
                            q[c * CH:(c + 1) * CH, :],
                            lhsT=xh_t[:, ks * P + c * CH:ks * P + (c + 1) * CH],
                            rhs=wt[0][HALF_KOFF[c] + ks][:, nf * NF:(nf + 1) * NF],
                            start=(ks == 0),
                            stop=(ks == 7),
                            tile_position=(0, c * CH),
                        )
                dst = o_h[:, nf * NF:(nf + 1) * NF]
                if nf % 2:
                    nc.vector.tensor_scalar_mul(dst, q[:], g2[:, 4:5])
                else:
                    nc.scalar.activation(
                        dst, q[:],
                        mybir.ActivationFunctionType.Copy,
                        scale=g2[:, 4:5],
                    )
                if nf == 1:
                    dma_out(NFULL, o_h, 0)
                elif nf == 3:
                    dma_out(NFULL, o_h, 1)

            # phase 2: tiles 2+3 on slice B.
            open_tile(2)
            open_tile(3)
            phase(2, 3)
    nc.finalize()
    return nc


def _build_program_fallback(cap: int):
    import concourse.mybir as mybir
    import concourse.tile as tile

    f16 = mybir.dt.float16
    f32 = mybir.dt.float32
    ntok = cap // P

    nc = _new_bacc()
    xT = nc.declare_dram_parameter("xT", [KSUB, P, cap], f16, isOutput=False)
    wk = nc.declare_dram_parameter("wk", [KSUB, P, H], f16, isOutput=False)
    gs = nc.declare_dram_parameter("gs", [P, ntok], f32, isOutput=False)
    ho = nc.declare_dram_parameter("ho", [ntok, P, H], f16, isOutput=True)

    with tile.TileContext(nc) as tc:
        with tc.tile_pool(name="sb", bufs=1) as sb, \
             tc.tile_pool(name="ps", bufs=2, space="PSUM") as psp:
            xt, wt = [], []
            for k in range(KSUB):
                x_t = sb.tile([P, cap], f16, name=f"x{k}", tag=f"x{k}", bufs=1)
                nc.sync.dma_start(x_t[:], xT[k, :, :])
                w_t = sb.tile([P, H], f16, name=f"w{k}", tag=f"w{k}", bufs=1)
                nc.sync.dma_start(w_t[:], wk[k, :, :])
                xt.append(x_t)
                wt.append(w_t)
            g_raw = sb.tile([P, ntok], f32, name="g_raw", tag="g_raw", bufs=1)
            nc.sync.dma_start(g_raw[:], gs[:, :])
            g2 = sb.tile([P, ntok], f32, name="g2", tag="g2", bufs=1)
            nc.scalar.copy(g2[:], g_raw[:])

            for t in range(ntok):
                ps = psp.tile([P, H], f32, name=f"ps{t}", tag="ps", bufs=2)
                for k in range(KSUB):
                    lhs = xt[k][:, t * P:(t + 1) * P]
                    for h in range(NH):
                        nc.tensor.matmul(
                            ps[:, h * NF:(h + 1) * NF],
                            lhsT=lhs,
                            rhs=wt[k][:, h * NF:(h + 1) * NF],
                            start=(k == 0),
                            stop=(k == KSUB - 1),
                        )
                o_t = sb.tile([P, H], f16, name=f"o{t}", tag="o", bufs=ntok)
                nc.scalar.activation(
                    o_t[:],
                    ps[:],
                    mybir.ActivationFunctionType.Copy,
                    scale=g2[:, t:t + 1],
                )
                nc.sync.dma_start(ho[t, :, :], o_t[:])
    nc.finalize()
    return nc


def _get_program(key):
    if key not in _prog_cache:
        if key == "tiles":
            _prog_cache[key] = _build_program_tiles()
        else:
            _prog_cache[key] = _build_program_fallback(int(key.split(":")[1]))
    return _prog_cache[key]


def _route(logits, topk):
    """numpy replica of jax.lax.top_k + softmax over selected logits."""
    idx = np.argsort(-logits, axis=-1, kind="stable")[:, :topk]      # [T, topk]
    vals = np.take_along_axis(logits, idx, axis=-1)
    mx = vals.max(-1, keepdims=True)
    gate = np.exp(vals - mx)
    gate = gate / gate.sum(-1, keepdims=True)                        # f32
    return idx, gate


def _pair_groups(chunk_counts):
    """Pair the 16 (e, kh) groups onto 8 cores: i-th largest with i-th
    smallest.  Returns [(groupA, groupB)] or None if some pair exceeds the
    compiled (CAP_A, CAP_B) chunk-slot capacity."""
    groups = []
    for e, n in enumerate(chunk_counts):
        for kh in range(2):
            groups.append((int(n), e, kh))
    groups.sort(reverse=True)
    pairs = []
    for i in range(N_CORES):
        ga, gb = groups[i], groups[15 - i]
        if ga[0] > CAP_A or gb[0] > CAP_B:
            return None
        pairs.append((ga, gb))
    return pairs


def prepare(inputs):
    """Host routing + per-core input construction.

    Returns (nc, launches, combine): launches is a list of per-launch in_maps
    (one dict per core); combine(list_of_per_launch_results) -> final output.
    """
    x = np.asarray(inputs["intermediate_states"])          # [R, TK, I_PR] f16
    w = np.asarray(inputs["w"])                            # [R, E, I_PR, H] f16
    logits = np.asarray(inputs["router_logits"]).astype(np.float32)  # [T, E]
    topk = int(np.asarray(inputs["topk"]))

    T, E_ = logits.shape
    TK = T * topk
    assert x.shape == (R, TK, I_PR) and w.shape == (R, E_, I_PR, H) and E_ == E

    idx, gate = _route(logits, topk)
    flat_e = idx.reshape(-1)                               # expert of tk
    counts = np.bincount(flat_e, minlength=E)
    starts = np.zeros(E + 1, np.int64)
    starts[1:] = np.cumsum(counts)
    order = np.argsort(flat_e, kind="stable")              # tks sorted by expert
    g_flat = gate.reshape(TK)
    xf = np.ascontiguousarray(x.transpose(1, 0, 2)).reshape(TK, K)  # [TK, 4096]

    chunk_counts = [-(-int(c) // CH) for c in counts]
    pairs = _pair_groups(chunk_counts)
    if pairs is not None:
        return _prepare_tiles(w, xf, g_flat, order, starts, pairs, topk, T)
    return _prepare_fallback(w, xf, g_flat, order, starts, counts, topk, T)


# chunk-slot order per W slice: (tile, col) positions; A overflows into the
# half tile (2 chunks, k-split across column pairs)
A_SLOTS = [(0, 0), (0, 1), (0, 2), (0, 3), (1, 0), (1, 1), (1, 2), (1, 3)]
B_SLOTS = [(2, 0), (2, 1), (2, 2), (2, 3), (3, 0), (3, 1), (3, 2), (3, 3)]


def _prepare_tiles(w, xf, g_flat, order, starts, pairs, topk, T):
    TK = T * topk
    nc = _get_program("tiles")

    xjs = np.zeros((N_CORES, NFULL, P, KS2, P), np.float16)
    xhs = np.zeros((N_CORES, P, 8, P), np.float16)
    whs = np.zeros((N_CORES, 2, KS2, P, H), np.float16)
    gss = np.zeros((N_CORES, P, NTILE), np.float32)
    # pos[plane, tk] = row index of tk's partial in the assembled h rows;
    # planes 0/1 = kh 0/1 main partial, 2/3 = kh 0/1 half-tile second half.
    ZROW = N_CORES * NTILE * P
    pos = np.full((4, TK), ZROW, np.int64)

    for core, (ga, gb) in enumerate(pairs):
        for s, (nch, e, kh) in enumerate((ga, gb)):
            if nch == 0:
                continue
            toks_e = order[starts[e]:starts[e + 1]]
            whs[core, s] = np.ascontiguousarray(
                w[2 * kh:2 * kh + 2, e].reshape(KH, H)).reshape(KS2, P, H)
            slots = A_SLOTS if s == 0 else B_SLOTS
            for j in range(nch):
                toks = toks_e[j * CH:(j + 1) * CH]
                n = len(toks)
                xs = xf[toks, kh * KH:(kh + 1) * KH]       # [n, 2048] f16
                blk = xs.reshape(n, KS2, P).transpose(2, 1, 0)  # [P, ks, n]
                if j < len(slots):
                    t, c = slots[j]
                    xjs[core, t, :, :, c * CH:c * CH + n] = blk
                    gss[core, c * CH:c * CH + n, t] = g_flat[toks]
                    pos[kh, toks] = (core * NTILE + t) * P + c * CH + np.arange(n)
                else:
                    # half tile: chunk split into (ks 0-7, ks 8-15) columns
                    jj = j - len(slots)
                    assert s == 0 and jj < 2
                    for h in range(2):
                        c = 2 * jj + h
                        xhs[core, :, :, c * CH:c * CH + n] = blk[:, 8 * h:8 * (h + 1), :]
                        gss[core, c * CH:c * CH + n, NFULL] = g_flat[toks]
                        pos[2 * h + kh, toks] = \
                            (core * NTILE + NFULL) * P + c * CH + np.arange(n)

    launches = [[{"xj": xjs[c].reshape(NFULL, P, KS2 * P),
                  "xh": xhs[c].reshape(P, 8 * P),
                  "wh": whs[c], "gs": gss[c]} for c in range(N_CORES)]]

    def combine(all_results):
        res = all_results[0]
        h_all = np.concatenate(
            [res[c]["ho"].reshape(NTILE * P, H) for c in range(N_CORES)]
            + [np.zeros((1, H), np.float16)], axis=0)
        y = np.zeros((T, H), np.float32)
        for plane in range(4):
            for kk in range(topk):
                y += h_all[pos[plane, kk::topk]].astype(np.float32)
        return y.astype(np.float16).reshape(R, T // R, H)

    return nc, launches, combine


def _prepare_fallback(w, xf, g_flat, order, starts, counts, topk, T):
    TK = T * topk
    cap_needed = -(-max(int(counts.max()), 1) // P) * P
    cap_launch = min(cap_needed, CAP_FB)
    n_launch = -(-cap_needed // cap_launch)
    cap_total = n_launch * cap_launch
    ntok_l = cap_launch // P

    nc = _get_program(f"fb:{cap_launch}")

    pos = np.empty(TK, np.int64)
    for e in range(E):
        toks = order[starts[e]:starts[e + 1]]
        pos[toks] = e * cap_total + np.arange(len(toks))

    launches = []
    for j in range(n_launch):
        in_maps = []
        for e in range(E):
            toks = order[starts[e]:starts[e + 1]][j * cap_launch:(j + 1) * cap_launch]
            c = len(toks)
            xTe = np.zeros((K, cap_launch), np.float16)
            gse = np.zeros((cap_launch,), np.float32)
            if c:
                xTe[:, :c] = xf[toks].T
                gse[:c] = g_flat[toks]
            in_maps.append({
                "xT": np.ascontiguousarray(xTe.reshape(KSUB, P, cap_launch)),
                "wk": np.ascontiguousarray(w[:, e].reshape(K, H)).reshape(KSUB, P, H),
                "gs": np.ascontiguousarray(gse.reshape(ntok_l, P).T),
            })
        launches.append(in_maps)

    def combine(all_results):
        h_all = np.empty((E * cap_total, H), np.float16)
        for j, res in enumerate(all_results):
            for e in range(E):
                h_all[e * cap_total + j * cap_launch:
                      e * cap_total + (j + 1) * cap_launch] = \
                    res[e]["ho"].reshape(cap_launch, H)
        y = h_all[pos[0::topk]].astype(np.float32)
        for kk in range(1, topk):
            y += h_all[pos[kk::topk]].astype(np.float32)
        return y.astype(np.float16).reshape(R, T // R, H)

    return nc, launches, combine


def kernel(**inputs) -> np.ndarray:
    nc, launches, combine = prepare(inputs)
    from concourse.bass_utils import run_bass_kernel_spmd

    all_results = []
    for in_maps in launches:
        res = run_bass_kernel_spmd(nc, in_maps, core_ids=list(range(N_CORES)))
        all_results.append(res.results)
    return combine(all_results)


# revision 24
# speedup vs baseline: 1.3437x; 1.0532x over previous
"""MoE top-k routing + grouped down-proj GEMM + reduce-scatter for trn2 (8 cores).

Problem: intermediate_states [4, 2048, 1024] f16 (rank-sharded expanded-token
activations), w [4, 8, 1024, 2048] f16 (rank-sharded per-expert down-proj),
router_logits [1024, 8] f32, topk=2.  Output [4, 256, 2048] f16.

Strategy: per expanded token tk routed to expert e(tk):
y_part[tk] = gate(tk) * (x_full[tk] @ w_full[e(tk)]) with x_full [TK, 4096]
(rank dim folded into the contraction) and w_full[e] [4096, 2048].

Work is decomposed into (expert, K-half) groups; a group's tokens are split
into 32-token chunks (sum over groups = ~136 chunks for balanced routing vs
160 128-token-padded quarters in a 5-job layout).  Each core holds two W
slices (A, B: one (expert, khalf) [2048, 2048] f16 block each) and runs
4 full tiles + 1 half tile of PE work using 4x column tiling
(tile_size=(128, 32)): each tile issues 4 concurrent matmuls per (ks, nf)
group, one per 32-token column chunk, each streaming its own W slice.
Column capacity per core: 10 chunk-slots on slice A, 8 on slice B
(the half tile carries 2 B-chunks split into (ks 0-7)/(ks 8-15) halves).
The host pairs the 16 (e, kh) groups onto cores (largest with smallest),
which fits whenever the largest group is <= 10 chunks and the 9th largest
is <= 8.  PE work per core: 4.5 tile-equivalents (= 288 4-way column
groups + half-tile) ~= 62-66 us vs 69 us for the 5-job layout.

Each chunk accumulates fp32 in its own PSUM quarter region over its k-run
and gets its fp32 routing gate applied as a per-partition scale at PSUM
eviction (scalar engine for nf 0-1, vector for nf 2-3).  The final (half)
tile runs nf-outer so its quarters evict pipelined, shrinking the kernel
tail.  No collective: the host sums each token's partial rows.

Fallback: pathological routing (largest group > 10 chunks etc.) uses an
expert-per-core kernel (full K=4096, capacity padded to 128).
"""

import numpy as np

R, T_TOK, TOPK, E = 4, 1024, 2, 8
I_PR, H = 1024, 2048
K = R * I_PR            # 4096 contraction
P = 128
NF = 512                # matmul free-dim (one PSUM bank of fp32)
NH = H // NF            # 4
N_CORES = 8

KH = K // 2             # 2048 per K-half
KS2 = KH // P           # 16 k-subtiles per K-half
CH = 32                 # token chunk granularity (column-tile width)
NFULL = 4               # full tiles per core (+1 half tile)
NTILE = NFULL + 1
CAP_A, CAP_B = 10, 8    # chunk-slot capacity per W slice
# compiled tile -> W slice map for the 4 full tiles (0=A, 1=B); the half
# tile is all slice A and runs between phase 1 (t0/t1) and phase 2 (t2/t3),
# which delays the first wB consumption by the half tile's span.
SLICE_OF = (0, 0, 1, 1)
# half tile: 4 column slots = 2 A-chunks split into k-halves
HALF_KOFF = (0, 8, 0, 8)
NWARM = 24

# fallback (expert-per-core) mode
KSUB = K // P           # 32
CAP_FB = 384            # token capacity per launch in fallback mode

_prog_cache: dict[str, object] = {}


def _new_bacc():
    from concourse import bacc

    return bacc.Bacc(
        "TRN2",
        target_bir_lowering=False,
        debug=False,
        num_devices=N_CORES,
    )


def _build_program_tiles():
    import concourse.mybir as mybir
    import concourse.tile as tile

    f16 = mybir.dt.float16
    f32 = mybir.dt.float32

    nc = _new_bacc()
    # xj[t, p, ks*P + c*CH + m] = x value of tile-t column-chunk c token m at
    # K-row ks*P + p of the chunk's K-half: the SBUF stationary layout.
    xj = nc.declare_dram_parameter("xj", [NFULL, P, KS2 * P], f16, isOutput=False)
    xh = nc.declare_dram_parameter("xh", [P, 8 * P], f16, isOutput=False)
    wh = nc.declare_dram_parameter("wh", [2, KS2, P, H], f16, isOutput=False)
    gs = nc.declare_dram_parameter("gs", [P, NTILE], f32, isOutput=False)
    ho = nc.declare_dram_parameter("ho", [NTILE, P, H], f16, isOutput=True)

    with tile.TileContext(nc) as tc:
        with tc.tile_pool(name="sb", bufs=1) as sb, \
             tc.tile_pool(name="ps", bufs=2, space="PSUM") as psp:
            xt = [sb.tile([P, KS2 * P], f16, name=f"x{t}", tag=f"x{t}", bufs=1)
                  for t in range(NFULL)]
            xh_t = sb.tile([P, 8 * P], f16, name="xh", tag="xh", bufs=1)
            wt = [[sb.tile([P, H], f16, name=f"w{s}_{ks}", tag=f"w{s}_{ks}",
                           bufs=1) for ks in range(KS2)] for s in range(2)]
            g_raw = sb.tile([P, NTILE], f32, name="g_raw", tag="g_raw", bufs=1)

            HXB = KS2 * P // 2  # half of a full x tile's free dim

            def dma_x(t, half):
                sl = slice(half * HXB, (half + 1) * HXB)
                nc.sync.dma_start(xt[t][:, sl], xj[t, :, sl])

            def dma_w(s, ks, eng=None):
                (eng or nc.sync).dma_start(wt[s][ks][:], wh[s, ks, :, :])

            # All bulk input DMAs ride the sync HW queue in consumption
            # order (a second busy queue lifts aggregate DMA to ~360 GB/s
            # but power-throttles the PE clock to ~2.0 GHz — a net loss).
            # Only three small late-needed x transfers go to the scalar
            # queue to keep the sync ring lean.  wA0 is split in halves so
            # the first matmul group starts as early as possible.
            HWB = H // 2

            QXB = KS2 * P // 4  # quarter of a full x tile's free dim

            def dma_xq(t, quarter):
                sl = slice(quarter * QXB, (quarter + 1) * QXB)
                nc.sync.dma_start(xt[t][:, sl], xj[t, :, sl])

            nc.sync.dma_start(wt[0][0][:, :HWB], wh[0, 0, :, :HWB])
            dma_xq(0, 0)
            nc.sync.dma_start(wt[0][0][:, HWB:], wh[0, 0, :, HWB:])
            dma_xq(1, 0)
            nc.sync.dma_start(g_raw[:], gs[:, :])
            dma_w(0, 1)
            dma_w(0, 2)
            dma_xq(0, 1)
            dma_xq(1, 1)
            dma_w(0, 3)
            dma_w(0, 4)
            dma_xq(0, 2)
            dma_xq(1, 2)
            dma_w(0, 5)
            dma_w(0, 6)
            dma_xq(0, 3)
            dma_xq(1, 3)
            for ks in range(7, KS2):
                dma_w(0, ks)
            dma_x(2, 0)
            dma_x(3, 0)
            for ks in range(KS2):
                dma_w(1, ks)

            # gates: a scalar-engine copy of g_raw; evictions read the copy
            # so their gate dependency is ACT-engine-local.
            g2 = sb.tile([P, NTILE], f32, name="g2", tag="g2", bufs=1)
            nc.scalar.copy(g2[:], g_raw[:])
            # three small late-needed x transfers ride the (otherwise idle)
            # scalar HW queue; issued after the g2 copy so they don't steal
            # DMA bandwidth from the critical first W chunks.
            # tiny read of a mid-stream W chunk pins these issues until the
            # A stream's critical window has drained
            scr = sb.tile([1, 1], f16, name="scr", tag="scr", bufs=1)
            nc.scalar.copy(scr[:], wt[0][8][:1, :1])
            nc.scalar.dma_start(xh_t[:], xh[:, :])
            sl31 = slice(HXB, 2 * HXB)
            nc.scalar.dma_start(xt[2][:, sl31], xj[2, :, sl31])
            nc.scalar.dma_start(xt[3][:, sl31], xj[3, :, sl31])

            psq = {}

            def open_tile(t):
                psq[t] = [psp.tile([P, NF], f32, name=f"ps{t}_{nf}", tag="ps",
                                   bufs=8) for nf in range(NH)]

            open_tile(0)
            open_tile(1)

            # HAM warmup in the same (128, 32) tile mode as the real matmuls:
            # keeps the PE busy while the first DMAs stream in; garbage goes
            # to tile 0's first PSUM quarter, cleared by the first real
            # start=True matmul.
            warm_in = sb.tile([P, P], f16, name="warm_in", tag="warm", bufs=1)
            nc.vector.memset(warm_in[:], 0.0)
            for i in range(NWARM):
                nc.tensor.matmul(
                    psq[0][0][:, 0:P],
                    lhsT=warm_in[:],
                    rhs=warm_in[:],
                    start=(i == 0),
                    stop=(i == NWARM - 1),
                )

            # full tiles: all 4 column chunks share one W slice, so each
            # (ks, nf) step is a single M=128 matmul (plain 128x128 mode).
            def mm_group(t, ks):
                s = SLICE_OF[t]
                for nf in range(NH):
                    nc.tensor.matmul(
                        psq[t][nf][:, :],
                        lhsT=xt[t][:, ks * P:(ks + 1) * P],
                        rhs=wt[s][ks][:, nf * NF:(nf + 1) * NF],
                        start=(ks == 0),
                        stop=(ks == KS2 - 1),
                    )

            def evict_quarter(t, nf, o_t):
                dst = o_t[:, nf * NF:(nf + 1) * NF]
                src = psq[t][nf][:]
                # alternate engines per quarter so each output half (two
                # quarters) is ready one eviction-latency after the stop
                if nf % 2:
                    nc.vector.tensor_scalar_mul(dst, src, g2[:, t:t + 1])
                else:
                    nc.scalar.activation(
                        dst, src,
                        mybir.ActivationFunctionType.Copy,
                        scale=g2[:, t:t + 1],
                    )

            def dma_out(t, o_t, half):
                nc.sync.dma_start(
                    ho[t, :, half * (H // 2):(half + 1) * (H // 2)],
                    o_t[:, half * (H // 2):(half + 1) * (H // 2)])

            def evict_pair(ta, tb):
                o_a = sb.tile([P, H], f16, name=f"o{ta}", tag="o", bufs=NTILE)
                o_b = sb.tile([P, H], f16, name=f"o{tb}", tag="o", bufs=NTILE)
                # ta stopped first (de-interleaved phase tail); pipeline both
                # engines and the output DMAs per half.
                evict_quarter(ta, 0, o_a)
                evict_quarter(ta, 1, o_a)
                dma_out(ta, o_a, 0)
                evict_quarter(ta, 2, o_a)
                evict_quarter(ta, 3, o_a)
                dma_out(ta, o_a, 1)
                evict_quarter(tb, 0, o_b)
                evict_quarter(tb, 1, o_b)
                dma_out(tb, o_b, 0)
                evict_quarter(tb, 2, o_b)
                evict_quarter(tb, 3, o_b)
                dma_out(tb, o_b, 1)

            def phase(ta, tb):
                # interleaved per k-subtile (the pair consumes each arriving
                # W chunk over ~1.8us, matching the DMA stream rate); the
                # last two k-subtiles de-interleave so ta's eviction overlaps
                # tb's final matmuls.
                for ks in range(KS2 - 2):
                    for t in (ta, tb):
                        mm_group(t, ks)
                for t in (ta, tb):
                    for ks in (KS2 - 2, KS2 - 1):
                        mm_group(t, ks)
                evict_pair(ta, tb)

            # phase 1: tiles 0+1 on slice A.
            phase(0, 1)

            # phase 1.5: half tile on resident slice A (its span defers the
            # first wB consumption, buying the B stream ~7us of headroom).
            # nf-outer so its quarters evict pipelined.
            o_h = sb.tile([P, H], f16, name="o4", tag="o", bufs=NTILE)
            for nf in range(NH):
                q = psp.tile([P, NF], f32, name=f"ps4_{nf}", tag="ps", bufs=8)
                for ks in range(8):
                    for c in range(4):
                        nc.tensor.matmul(
                            q[c * CH:(c + 1) * CH, :],
                            lhsT=xh_t[:, ks * P + c * CH:ks * P + (c + 1) * CH],
                            rhs=wt[0][HALF_KOFF[c] + ks][:, nf * NF:(nf + 1) * NF],
                            start=(ks == 0),
                            stop=(ks == 7),
                            tile_position=(0, c * CH),
                        )
                dst = o_h[:, nf * NF:(nf + 1) * NF]
                if nf % 2:
                    nc.vector.tensor_scalar_mul(dst, q[:], g2[:, 4:5])
                else:
                    nc.scalar.activation(
                        dst, q[:],
                        mybir.ActivationFunctionType.Copy,
                        scale=g2[:, 4:5],
                    )
                if nf == 1:
                    dma_out(NFULL, o_h, 0)
                elif nf == 3:
                    dma_out(NFULL, o_h, 1)

            # phase 2: tiles 2+3 on slice B.
            open_tile(2)
            open_tile(3)
            phase(2, 3)
    nc.finalize()
    return nc


def _build_program_fallback(cap: int):
    import concourse.mybir as mybir
    import concourse.tile as tile

    f16 = mybir.dt.float16
    f32 = mybir.dt.float32
    ntok = cap // P

    nc = _new_bacc()
    xT = nc.declare_dram_parameter("xT", [KSUB, P, cap], f16, isOutput=False)
    wk = nc.declare_dram_parameter("wk", [KSUB, P, H], f16, isOutput=False)
    gs = nc.declare_dram_parameter("gs", [P, ntok], f32, isOutput=False)
    ho = nc.declare_dram_parameter("ho", [ntok, P, H], f16, isOutput=True)

    with tile.TileContext(nc) as tc:
        with tc.tile_pool(name="sb", bufs=1) as sb, \
             tc.tile_pool(name="ps", bufs=2, space="PSUM") as psp:
            xt, wt = [], []
            for k in range(KSUB):
                x_t = sb.tile([P, cap], f16, name=f"x{k}", tag=f"x{k}", bufs=1)
                nc.sync.dma_start(x_t[:], xT[k, :, :])
                w_t = sb.tile([P, H], f16, name=f"w{k}", tag=f"w{k}", bufs=1)
                nc.sync.dma_start(w_t[:], wk[k, :, :])
                xt.append(x_t)
                wt.append(w_t)
            g_raw = sb.tile([P, ntok], f32, name="g_raw", tag="g_raw", bufs=1)
            nc.sync.dma_start(g_raw[:], gs[:, :])
            g2 = sb.tile([P, ntok], f32, name="g2", tag="g2", bufs=1)
            nc.scalar.copy(g2[:], g_raw[:])

            for t in range(ntok):
                ps = psp.tile([P, H], f32, name=f"ps{t}", tag="ps", bufs=2)
                for k in range(KSUB):
                    lhs = xt[k][:, t * P:(t + 1) * P]
                    for h in range(NH):
                        nc.tensor.matmul(
                            ps[:, h * NF:(h + 1) * NF],
                            lhsT=lhs,
                            rhs=wt[k][:, h * NF:(h + 1) * NF],
                            start=(k == 0),
                            stop=(k == KSUB - 1),
                        )
                o_t = sb.tile([P, H], f16, name=f"o{t}", tag="o", bufs=ntok)
                nc.scalar.activation(
                    o_t[:],
                    ps[:],
                    mybir.ActivationFunctionType.Copy,
                    scale=g2[:, t:t + 1],
                )
                nc.sync.dma_start(ho[t, :, :], o_t[:])
    nc.finalize()
    return nc


def _get_program(key):
    if key not in _prog_cache:
        if key == "tiles":
            _prog_cache[key] = _build_program_tiles()
        else:
            _prog_cache[key] = _build_program_fallback(int(key.split(":")[1]))
    return _prog_cache[key]


def _route(logits, topk):
    """numpy replica of jax.lax.top_k + softmax over selected logits."""
    idx = np.argsort(-logits, axis=-1, kind="stable")[:, :topk]      # [T, topk]
    vals = np.take_along_axis(logits, idx, axis=-1)
    mx = vals.max(-1, keepdims=True)
    gate = np.exp(vals - mx)
    gate = gate / gate.sum(-1, keepdims=True)                        # f32
    return idx, gate


def _pair_groups(chunk_counts):
    """Pair the 16 (e, kh) groups onto 8 cores: i-th largest with i-th
    smallest.  Returns [(groupA, groupB)] or None if some pair exceeds the
    compiled (CAP_A, CAP_B) chunk-slot capacity."""
    groups = []
    for e, n in enumerate(chunk_counts):
        for kh in range(2):
            groups.append((int(n), e, kh))
    groups.sort(reverse=True)
    pairs = []
    for i in range(N_CORES):
        ga, gb = groups[i], groups[15 - i]
        if ga[0] > CAP_A or gb[0] > CAP_B:
            return None
        pairs.append((ga, gb))
    return pairs


def prepare(inputs):
    """Host routing + per-core input construction.

    Returns (nc, launches, combine): launches is a list of per-launch in_maps
    (one dict per core); combine(list_of_per_launch_results) -> final output.
    """
    x = np.asarray(inputs["intermediate_states"])          # [R, TK, I_PR] f16
    w = np.asarray(inputs["w"])                            # [R, E, I_PR, H] f16
    logits = np.asarray(inputs["router_logits"]).astype(np.float32)  # [T, E]
    topk = int(np.asarray(inputs["topk"]))

    T, E_ = logits.shape
    TK = T * topk
    assert x.shape == (R, TK, I_PR) and w.shape == (R, E_, I_PR, H) and E_ == E

    idx, gate = _route(logits, topk)
    flat_e = idx.reshape(-1)                               # expert of tk
    counts = np.bincount(flat_e, minlength=E)
    starts = np.zeros(E + 1, np.int64)
    starts[1:] = np.cumsum(counts)
    order = np.argsort(flat_e, kind="stable")              # tks sorted by expert
    g_flat = gate.reshape(TK)
    xf = np.ascontiguousarray(x.transpose(1, 0, 2)).reshape(TK, K)  # [TK, 4096]

    chunk_counts = [-(-int(c) // CH) for c in counts]
    pairs = _pair_groups(chunk_counts)
    if pairs is not None:
        return _prepare_tiles(w, xf, g_flat, order, starts, pairs, topk, T)
    return _prepare_fallback(w, xf, g_flat, order, starts, counts, topk, T)


# chunk-slot order per W slice: (tile, col) positions; A overflows into the
# half tile (2 chunks, k-split across column pairs)
A_SLOTS = [(0, 0), (0, 1), (0, 2), (0, 3), (1, 0), (1, 1), (1, 2), (1, 3)]
B_SLOTS = [(2, 0), (2, 1), (2, 2), (2, 3), (3, 0), (3, 1), (3, 2), (3, 3)]


def _prepare_tiles(w, xf, g_flat, order, starts, pairs, topk, T):
    TK = T * topk
    nc = _get_program("tiles")

    xjs = np.zeros((N_CORES, NFULL, P, KS2, P), np.float16)
    xhs = np.zeros((N_CORES, P, 8, P), np.float16)
    whs = np.zeros((N_CORES, 2, KS2, P, H), np.float16)
    gss = np.zeros((N_CORES, P, NTILE), np.float32)
    # pos[plane, tk] = row index of tk's partial in the assembled h rows;
    # planes 0/1 = kh 0/1 main partial, 2/3 = kh 0/1 half-tile second half.
    ZROW = N_CORES * NTILE * P
    pos = np.full((4, TK), ZROW, np.int64)

    for core, (ga, gb) in enumerate(pairs):
        for s, (nch, e, kh) in enumerate((ga, gb)):
            if nch == 0:
                continue
            toks_e = order[starts[e]:starts[e + 1]]
            whs[core, s] = np.ascontiguousarray(
                w[2 * kh:2 * kh + 2, e].reshape(KH, H)).reshape(KS2, P, H)
            slots = A_SLOTS if s == 0 else B_SLOTS
            for j in range(nch):
                toks = toks_e[j * CH:(j + 1) * CH]
                n = len(toks)
                xs = xf[toks, kh * KH:(kh + 1) * KH]       # [n, 2048] f16
                blk = xs.reshape(n, KS2, P).transpose(2, 1, 0)  # [P, ks, n]
                if j < len(slots):
                    t, c = slots[j]
                    xjs[core, t, :, :, c * CH:c * CH + n] = blk
                    gss[core, c * CH:c * CH + n, t] = g_flat[toks]
                    pos[kh, toks] = (core * NTILE + t) * P + c * CH + np.arange(n)
                else:
                    # half tile: chunk split into (ks 0-7, ks 8-15) columns
                    jj = j - len(slots)
                    assert s == 0 and jj < 2
                    for h in range(2):
                        c = 2 * jj + h
                        xhs[core, :, :, c * CH:c * CH + n] = blk[:, 8 * h:8 * (h + 1), :]
                        gss[core, c * CH:c * CH + n, NFULL] = g_flat[toks]
                        pos[2 * h + kh, toks] = \
                            (core * NTILE + NFULL) * P + c * CH + np.arange(n)

    launches = [[{"xj": xjs[c].reshape(NFULL, P, KS2 * P),
                  "xh": xhs[c].reshape(P, 8 * P),
                  "wh": whs[c], "gs": gss[c]} for c in range(N_CORES)]]

    def combine(all_results):
        res = all_results[0]
        h_all = np.concatenate(
            [res[c]["ho"].reshape(NTILE * P, H) for c in range(N_CORES)]
            + [np.zeros((1, H), np.float16)], axis=0)
        y = np.zeros((T, H), np.float32)
        for plane in range(4):
            for kk in range(topk):
                y += h_all[pos[plane, kk::topk]].astype(np.float32)
        return y.astype(np.float16).reshape(R, T // R, H)

    return nc, launches, combine


def _prepare_fallback(w, xf, g_flat, order, starts, counts, topk, T):
    TK = T * topk
    cap_needed = -(-max(int(counts.max()), 1) // P) * P
    cap_launch = min(cap_needed, CAP_FB)
    n_launch = -(-cap_needed // cap_launch)
    cap_total = n_launch * cap_launch
    ntok_l = cap_launch // P

    nc = _get_program(f"fb:{cap_launch}")

    pos = np.empty(TK, np.int64)
    for e in range(E):
        toks = order[starts[e]:starts[e + 1]]
        pos[toks] = e * cap_total + np.arange(len(toks))

    launches = []
    for j in range(n_launch):
        in_maps = []
        for e in range(E):
            toks = order[starts[e]:starts[e + 1]][j * cap_launch:(j + 1) * cap_launch]
            c = len(toks)
            xTe = np.zeros((K, cap_launch), np.float16)
            gse = np.zeros((cap_launch,), np.float32)
            if c:
                xTe[:, :c] = xf[toks].T
                gse[:c] = g_flat[toks]
            in_maps.append({
                "xT": np.ascontiguousarray(xTe.reshape(KSUB, P, cap_launch)),
                "wk": np.ascontiguousarray(w[:, e].reshape(K, H)).reshape(KSUB, P, H),
                "gs": np.ascontiguousarray(gse.reshape(ntok_l, P).T),
            })
        launches.append(in_maps)

    def combine(all_results):
        h_all = np.empty((E * cap_total, H), np.float16)
        for j, res in enumerate(all_results):
            for e in range(E):
                h_all[e * cap_total + j * cap_launch:
                      e * cap_total + (j + 1) * cap_launch] = \
                    res[e]["ho"].reshape(cap_launch, H)
        y = h_all[pos[0::topk]].astype(np.float32)
        for kk in range(1, topk):
            y += h_all[pos[kk::topk]].astype(np.float32)
        return y.astype(np.float16).reshape(R, T // R, H)

    return nc, launches, combine


def kernel(**inputs) -> np.ndarray:
    nc, launches, combine = prepare(inputs)
    from concourse.bass_utils import run_bass_kernel_spmd

    all_results = []
    for in_maps in launches:
        res = run_bass_kernel_spmd(nc, in_maps, core_ids=list(range(N_CORES)))
        all_results.append(res.results)
    return combine(all_results)
